# revision 8
# baseline (speedup 1.0000x reference)
"""Distorted-SSIM loss kernel for Trainium2 (8 NeuronCores, data parallel).

v3 — engine-rebalanced + software-pipelined rewrite of v2 (919us).

Decomposition per [512,512] plane (x, y = img planes):
    Host precomputes 4 maps in fp16: S = x+y, D = x-y, U = x^2+y^2,
    V2 = 2xy (eliminates all on-chip prep work).
    After separable blur (col conv then row conv, both banded matmuls):
      sa = 0.5*Sb^2, sb = 0.5*Db^2          (ScalarE Act-Square from PSUM)
      a0 = sa - sb  (= 2 mu1 mu2)           (Pool)
      q0 = sa + sb  (= mu1^2 + mu2^2)       (Pool / DVE round-robin)
      num = (V2b + C2 - a0) * (a0 + C1)     (custom DVE op ND, PSUM src)
      den = (Ub  + C2 - q0) * (q0 + C1)     (custom DVE op ND, fp32 out)
      loss-col += num * recip_1nr(den)      (custom DVE op FMR: bitwise-NOT
                                             seed + 1 Newton step + mul +
                                             accumulate, one instruction)

Key changes vs v2:
  - inputs: 4 host-precomputed maps (S,D,U,V2) instead of (x,y); no
    on-chip w1/w2/xx/yy/up/vp2 passes.
  - fused FMR op: reciprocal + multiply + column-sum accumulate in one
    DVE pass (was reciprocal_approx_fast + scalar_tensor_tensor).
  - stage-1 PSUM retiled to 1-bank tiles (c-pairs) + one shared
    cross-map tail tile per u; PSUM = 3 + 1 + 4 banks = 8 exactly.
  - evictions split ScalarE/DVE, a0/q0 split Pool/DVE for engine balance.
  - stage-1 of plane p emission-interleaved with stage-2 of plane p-1 so
    the PE always has back-to-back work (HAM stays warm).
"""

import sys
import numpy as np

for _p in ("/opt/trn_rl_repo",):
    if _p not in sys.path:
        sys.path.insert(0, _p)

SIGMA = 1.5
C1 = 0.01**2
C2 = 0.03**2

STARTS = [0, 113, 231, 349, 467]
NCH = 5
KSZ = [min(128, 512 - s) for s in STARTS]
MSZ = [118, 118, 118, 118, 40]
N_PLANES = 12
FREE = NCH * 512  # 2560
NMAPS = 4
NTILES = 15 * N_PLANES  # loss columns (3 combos x 5 u per plane)
OUTW = 192  # padded

# engine-balance knobs (tuned from trace)
EV_DVE_EVERY = 7  # every k-th stage-1 evict goes to DVE instead of ScE
Q0_DVE_EVERY = 8  # every k-th q0 goes to DVE instead of Pool


def _gaussian(n, sigma=SIGMA):
    x = np.arange(n, dtype=np.float64)
    g = np.exp(-((x - n // 2) ** 2) / (2.0 * sigma**2))
    return (g / g.sum()).astype(np.float32)


def _norm_fp16_taps(g):
    """fp16 taps ULP-nudged so the fp64 sum is exactly 1.0."""
    t = g.astype(np.float16)
    for _ in range(500):
        td = t.astype(np.float64)
        err = td.sum() - 1.0
        if abs(err) < 2e-8:
            break
        bits = t.view(np.uint16).astype(np.int32) + (1 if err < 0 else -1)
        stepped = bits.astype(np.uint16).view(np.float16)
        delta = stepped.astype(np.float64) - td
        ad = np.abs(delta)
        ok = ad <= abs(err) * 1.000001
        i = int(np.argmax(np.where(ok, ad, -1.0))) if ok.any() else int(np.argmin(ad))
        t[i] = stepped[i]
    return t


def _wblocks(k):
    """Banded conv blocks [128, 5, 118]: W[kk, c, m] = g[in - out + pad]."""
    g = _norm_fp16_taps(_gaussian(k)).astype(np.float32)
    p = k // 2
    W = np.zeros((128, NCH, 118), np.float32)
    kk = np.arange(128)
    for c, s in enumerate(STARTS):
        m = np.arange(MSZ[c])
        j = (s + kk[:, None]) - (118 * c + m[None, :]) + p
        valid = (j >= 0) & (j < k) & (kk[:, None] < KSZ[c])
        W[:, c, : MSZ[c]][valid] = g[np.clip(j, 0, k - 1)][valid]
    return W


def _overlap_planes(pl):
    """[12, 512, 512] fp32 -> [12, 128, 5*512] fp16 overlapped h-window tiles."""
    t = np.zeros((N_PLANES, NCH, 128, 512), np.float32)
    for c, s in enumerate(STARTS):
        t[:, c, : KSZ[c], :] = pl[:, s : s + KSZ[c], :]
    return np.ascontiguousarray(
        t.transpose(0, 2, 1, 3).reshape(N_PLANES, 128, NCH * 512)
    ).astype(np.float16)


_PROGRAM = {}
_SSIM_OPS = {}

# Chebyshev-centred constants for the 1-NR fast reciprocal (seed interval
# [-4.5,-4] after the BITWISE_NOT exponent flip; s1 centres the 1-NR error
# band at +-0.17%).
_FMR_S0 = -0.23549792
_FMR_S1 = 2.0017324


def _register_ssim_ops():
    """Register two fused custom DVE ops:
       ND : out = (Src0 + s0 - Src1) * (Src1 + s1)
       FMR: out = Src1 * recip_1nr(Src0); accum_out = column sum of out
    Registration appends to the concourse custom-op table (free rows exist;
    the per-NEFF DVE table is built from used ops at compile time)."""
    if _SSIM_OPS:
        return _SSIM_OPS
    from operator import add as _add
    from concourse import dve_ops as DO
    from concourse.dve_spec import AluOp, Bin, Spec, Src0, Src1, C0, C1 as SC1
    from concourse.dve_uop import DveOpSpec

    def _register(name, spec):
        if name in DO._SUB_OPCODE_FOR_NAME:
            return next(o for o in DO.OPS if o.name == name)
        row = DO._CUSTOM_DVE_ROW_BASE + len(DO.OPS)
        assert row < 0x20, "custom DVE opcode rows exhausted"
        shas = {}
        for ver in ("v3", "v4"):
            tmp = DveOpSpec(
                name=name, opcode=row,
                uops=DO.lower(spec, ver=ver),
                rd1_en=DO.has_src1(spec),
            )
            shas[ver] = tmp.sha(ver)
        op = DO.DveOp(name, spec, subdim=False, uops_sha=shas)
        DO.OPS.append(op)
        DO.CUSTOM_DVE_SPECS[name] = spec
        DO._SUB_OPCODE_FOR_NAME[name] = row
        return op

    nd_spec = Spec(
        body=(Src0 + C0 - Src1) * (Src1 + SC1),
        reference=lambda in0, in1, s0, s1, imm2: (
            (in0.astype(np.float32) + s0 - in1) * (in1 + s1)
        ).astype(np.float32),
    )

    _nx = Bin(AluOp.BITWISE_NOT, Src0, Src0)
    _y0 = _nx * C0
    _y1 = _y0 * (SC1 - Src0 * _y0)

    def _ref_fmr(in0, in1, s0, s1, imm2):
        x = in0.astype(np.float32)
        nx = (~x.view(np.int32)).view(np.float32)
        y0 = nx * np.float32(s0)
        y1 = (y0 * (np.float32(s1) - x * y0)).astype(np.float32)
        b = (y1 * in1.astype(np.float32)).astype(np.float32)
        return b, b.reshape(b.shape[0], -1).sum(axis=-1, keepdims=True)

    fmr_spec = Spec(body=_y1 * Src1, accum=_add, reference=_ref_fmr)

    _SSIM_OPS["nd"] = _register("SSIM_ND_ANT", nd_spec)
    _SSIM_OPS["fmr"] = _register("SSIM_FMR_ANT", fmr_spec)
    return _SSIM_OPS


def _build_program():
    import concourse.bass as bass
    import concourse.mybir as mybir
    from concourse import bacc, tile

    f32 = mybir.dt.float32
    f16 = mybir.dt.float16
    Act = mybir.ActivationFunctionType

    ops = _register_ssim_ops()
    nd_op = ops["nd"]
    fmr_op = ops["fmr"]

    nc = bacc.Bacc(None, target_bir_lowering=False)
    in_d = nc.dram_tensor("sduv", [N_PLANES, 128, NMAPS * FREE], f16, kind="ExternalInput")
    wb_d = nc.dram_tensor("wb", [128, 2, NCH, 118], f16, kind="ExternalInput")
    out_d = nc.dram_tensor("out", [128, OUTW], f32, kind="ExternalOutput")

    SQH = float(np.sqrt(0.5))
    COMBOS = ((0, 1), (1, 0), (1, 1))  # (colblur tap, rowblur tap)

    ev_ctr = [0]  # stage-1 eviction round-robin counter
    q0_ctr = [0]  # q0 round-robin counter

    with tile.TileContext(nc) as tc:
        with (
            tc.tile_pool(name="const", bufs=1) as cpool,
            tc.tile_pool(name="inp", bufs=3) as ipool,
            tc.tile_pool(name="cm", bufs=2) as cmpool,
            tc.tile_pool(name="win", bufs=6) as wpool,
            tc.tile_pool(name="ps1", bufs=1, space="PSUM") as ps1pool,
            tc.tile_pool(name="tail", bufs=1, space="PSUM") as tailpool,
            tc.tile_pool(name="ps2", bufs=5, space="PSUM") as ps2pool,
        ):
            wb = cpool.tile([128, 2, NCH, 118], f16, tag="wb")
            nc.sync.dma_start(wb[:], wb_d[:])
            wr = [wb[:, 0], wb[:, 1]]
            ocols = cpool.tile([128, OUTW], f32, tag="ocols")
            nc.vector.memset(ocols[:], 0.0)

            # dummy matmul: absorb wb DMA wait on PE once
            dummy = ps2pool.tile([128, 512], f32, tag="ps2", name="dummy")
            nc.tensor.matmul(
                dummy[0:118, 0:118], wb[0:128, 0, 0, 0:118], wb[0:128, 0, 0, 0:118],
                start=True, stop=True,
            )

            inps = [None, None]  # per-plane input tiles (bufs=2 rotation)
            cms = [None, None]   # per-plane cm tiles

            def emit_stage1_map(p, u, mp, tail):
                """Column conv for map mp, W-window u, plane p."""
                inp = inps[p % 2]
                cm = cms[p % 2]
                Kw = KSZ[u]
                ws = STARTS[u]
                mbase = mp * FREE
                # [4 c-chunks, 2 taps, 118] padded to 128 so each matmul's
                # [2, 118] output sits in a 256-float half-bank (2 banks total)
                ps = ps1pool.tile([128, 4, 2, 128], f32, tag="ps1")
                for ci_, c in enumerate((0, 1, 2, 3)):
                    Kc = KSZ[c]
                    lhs = inp[0:Kc, mbase + 512 * c + ws : mbase + 512 * c + ws + Kw]
                    nc.tensor.matmul(
                        ps[0:Kw, ci_, :, 0:118],
                        lhs, wb[0:Kc, :, c, 0:118],
                        start=True, stop=True,
                    )
                # tail chunk c=4 (H rows 472..511) into the shared tail tile
                Kc = KSZ[4]
                lhs = inp[0:Kc, mbase + 512 * 4 + ws : mbase + 512 * 4 + ws + Kw]
                nc.tensor.matmul(
                    tail[0:Kw, mp, :, 0:40],
                    lhs, wb[0:Kc, :, 4, 0:40],
                    start=True, stop=True,
                )
                # evict all 4 c-chunks x 2 taps in one op
                dst = cm[0:Kw, mp, :, 512 * u : 512 * u + 472]
                dst = dst.rearrange("p t (c j) -> p t c j", c=4)
                src = ps[0:Kw, :, :, 0:118].transpose([0, 2, 1, 3])
                if ev_ctr[0] % EV_DVE_EVERY == EV_DVE_EVERY - 1:
                    nc.vector.tensor_copy(dst, src)
                else:
                    nc.scalar.copy(dst, src)
                ev_ctr[0] += 1

            def emit_stage1_tail_evict(p, u, tail):
                cm = cms[p % 2]
                Kw = KSZ[u]
                nc.scalar.copy(
                    cm[0:Kw, :, :, 512 * u + 472 : 512 * u + 512],
                    tail[0:Kw],
                )

            def emit_stage2_group(p, ci, u):
                """Row conv + window math for combo ci, W-band u, plane p."""
                cm = cms[p % 2]
                srctap, rowtap = COMBOS[ci]
                Kw, Mu = KSZ[u], MSZ[u]
                pss = []
                for mp in range(NMAPS):
                    ps = ps2pool.tile([128, 512], f32, tag="ps2", name="ps2")
                    nc.tensor.matmul(
                        ps[0:Mu, :],
                        wr[rowtap][0:Kw, u, 0:Mu],
                        cm[0:Kw, mp, srctap, 512 * u : 512 * u + 512],
                        start=True, stop=True,
                    )
                    pss.append(ps)
                S, D, Up, Vp = pss

                sa = wpool.tile([128, 512], f16, tag="sa")
                sb = wpool.tile([128, 512], f16, tag="sb")
                nc.scalar.activation(sa[0:Mu, :], S[0:Mu, :], Act.Square, scale=SQH)
                nc.scalar.activation(sb[0:Mu, :], D[0:Mu, :], Act.Square, scale=SQH)

                a0 = wpool.tile([128, 512], f16, tag="a0")
                q0 = wpool.tile([128, 512], f16, tag="q0")
                nc.gpsimd.tensor_sub(a0[0:Mu, :], sa[0:Mu, :], sb[0:Mu, :])
                if q0_ctr[0] % Q0_DVE_EVERY == Q0_DVE_EVERY - 1:
                    nc.vector.tensor_add(q0[0:Mu, :], sa[0:Mu, :], sb[0:Mu, :])
                else:
                    nc.gpsimd.tensor_add(q0[0:Mu, :], sa[0:Mu, :], sb[0:Mu, :])
                q0_ctr[0] += 1

                num = wpool.tile([128, 512], f16, tag="num")
                den = wpool.tile([128, 512], f32, tag="den")
                nc.vector._custom_dve(
                    nd_op, out=num[0:Mu, :], in0=Vp[0:Mu, :],
                    in1=a0[0:Mu, :], s0=C2, s1=C1)
                nc.vector._custom_dve(
                    nd_op, out=den[0:Mu, :], in0=Up[0:Mu, :],
                    in1=q0[0:Mu, :], s0=C2, s1=C1)

                # fused: scratch = num * recip_1nr(den); ocols[:,t] = its col sum
                t = p * 15 + ci * 5 + u
                scratch = wpool.tile([128, 512], f16, tag="scr")
                nc.vector._custom_dve(
                    fmr_op, out=scratch[0:Mu, :], in0=den[0:Mu, :],
                    in1=num[0:Mu, :], s0=_FMR_S0, s1=_FMR_S1,
                    accum_out=ocols[0:Mu, t : t + 1])

            # software pipeline: stage-1 of plane p interleaved with
            # stage-2 of plane p-1.
            for p in range(N_PLANES + 1):
                if p < N_PLANES:
                    inp = ipool.tile([128, NMAPS * FREE], f16, tag="inp")
                    nc.sync.dma_start(inp[:], in_d[p])
                    inps[p % 2] = inp
                    cms[p % 2] = cmpool.tile([128, NMAPS, 2, FREE], f16, tag="cm", name="cm")
                for u in range(NCH):
                    tail = (
                        tailpool.tile([128, NMAPS, 2, 40], f32, tag="tail", name="tail")
                        if p < N_PLANES else None
                    )
                    if p > 0:
                        emit_stage2_group(p - 1, 0, u)
                    if p < N_PLANES:
                        emit_stage1_map(p, u, 0, tail)
                        emit_stage1_map(p, u, 1, tail)
                    if p > 0:
                        emit_stage2_group(p - 1, 1, u)
                    if p < N_PLANES:
                        emit_stage1_map(p, u, 2, tail)
                        emit_stage1_map(p, u, 3, tail)
                    if p > 0:
                        emit_stage2_group(p - 1, 2, u)
                    if p < N_PLANES:
                        emit_stage1_tail_evict(p, u, tail)

            nc.sync.dma_start(out_d[:], ocols[:])

    nc.finalize()
    return nc


def _get_program():
    global _PROGRAM
    if not isinstance(_PROGRAM, dict):
        globals()["_PROGRAM"] = {}
    if "v3" not in _PROGRAM:
        _PROGRAM["v3"] = _build_program()
    return _PROGRAM["v3"]


def _make_in_maps(img1, img2):
    x = np.asarray(img1)[:, :3].astype(np.float32)
    y = np.asarray(img2)[:, :3].astype(np.float32)
    wb = np.stack([_wblocks(5), _wblocks(11)], axis=1).astype(np.float16)
    in_maps = []
    for i in range(8):
        xs = x[4 * i : 4 * i + 4].reshape(N_PLANES, 512, 512)
        ys = y[4 * i : 4 * i + 4].reshape(N_PLANES, 512, 512)
        mS = _overlap_planes(xs + ys)
        mD = _overlap_planes(xs - ys)
        mU = _overlap_planes(xs * xs + ys * ys)
        mV = _overlap_planes(2.0 * xs * ys)
        sduv = np.concatenate([mS, mD, mU, mV], axis=2)  # [12, 128, 4*2560]
        in_maps.append({"sduv": sduv, "wb": wb})
    return in_maps


def _reduce_results(res):
    total = 0.0
    for i in range(8):
        total += np.asarray(res[i]["out"]).astype(np.float64).sum()
    npix = 32 * 3 * 512 * 512
    return np.float32(total / npix / 3.0)


def kernel(img1, img2):
    from concourse.bass_utils import run_bass_kernel_spmd

    in_maps = _make_in_maps(img1, img2)
    nc = _get_program()
    res = run_bass_kernel_spmd(nc, in_maps, core_ids=list(range(8))).results
    return _reduce_results(res)


# revision 13
# speedup vs baseline: 1.0528x; 1.0528x over previous
"""Distorted-SSIM loss kernel for Trainium2 (8 NeuronCores, data parallel).

v3 — engine-rebalanced + software-pipelined rewrite of v2 (919us).

Decomposition per [512,512] plane (x, y = img planes):
    Host precomputes 4 maps in fp16: S = x+y, D = x-y, U = x^2+y^2,
    V2 = 2xy (eliminates all on-chip prep work).
    After separable blur (col conv then row conv, both banded matmuls):
      sa = 0.5*Sb^2, sb = 0.5*Db^2          (ScalarE Act-Square from PSUM)
      a0 = sa - sb  (= 2 mu1 mu2)           (Pool)
      q0 = sa + sb  (= mu1^2 + mu2^2)       (Pool / DVE round-robin)
      num = (V2b + C2 - a0) * (a0 + C1)     (custom DVE op ND, PSUM src)
      den = (Ub  + C2 - q0) * (q0 + C1)     (custom DVE op ND, fp32 out)
      loss-col += num * recip_1nr(den)      (custom DVE op FMR: bitwise-NOT
                                             seed + 1 Newton step + mul +
                                             accumulate, one instruction)

Key changes vs v2:
  - inputs: 4 host-precomputed maps (S,D,U,V2) instead of (x,y); no
    on-chip w1/w2/xx/yy/up/vp2 passes.
  - fused FMR op: reciprocal + multiply + column-sum accumulate in one
    DVE pass (was reciprocal_approx_fast + scalar_tensor_tensor).
  - stage-1 PSUM retiled to 1-bank tiles (c-pairs) + one shared
    cross-map tail tile per u; PSUM = 3 + 1 + 4 banks = 8 exactly.
  - evictions split ScalarE/DVE, a0/q0 split Pool/DVE for engine balance.
  - stage-1 of plane p emission-interleaved with stage-2 of plane p-1 so
    the PE always has back-to-back work (HAM stays warm).
"""

import sys
import numpy as np

for _p in ("/opt/trn_rl_repo",):
    if _p not in sys.path:
        sys.path.insert(0, _p)

SIGMA = 1.5
C1 = 0.01**2
C2 = 0.03**2

STARTS = [0, 113, 231, 349, 467]
NCH = 5
KSZ = [min(128, 512 - s) for s in STARTS]
MSZ = [118, 118, 118, 118, 40]
N_PLANES = 12
FREE = NCH * 512  # 2560
NMAPS = 4
NTILES = 15 * N_PLANES  # loss columns (3 combos x 5 u per plane)
OUTW = 192  # padded

# engine-balance knobs (tuned from trace)
EV_DVE_EVERY = 3  # every k-th stage-1 evict goes to DVE instead of ScE
Q0_DVE_EVERY = 999  # every k-th q0 goes to DVE instead of Pool


def _gaussian(n, sigma=SIGMA):
    x = np.arange(n, dtype=np.float64)
    g = np.exp(-((x - n // 2) ** 2) / (2.0 * sigma**2))
    return (g / g.sum()).astype(np.float32)


def _norm_fp16_taps(g):
    """fp16 taps ULP-nudged so the fp64 sum is exactly 1.0."""
    t = g.astype(np.float16)
    for _ in range(500):
        td = t.astype(np.float64)
        err = td.sum() - 1.0
        if abs(err) < 2e-8:
            break
        bits = t.view(np.uint16).astype(np.int32) + (1 if err < 0 else -1)
        stepped = bits.astype(np.uint16).view(np.float16)
        delta = stepped.astype(np.float64) - td
        ad = np.abs(delta)
        ok = ad <= abs(err) * 1.000001
        i = int(np.argmax(np.where(ok, ad, -1.0))) if ok.any() else int(np.argmin(ad))
        t[i] = stepped[i]
    return t


def _wblocks(k):
    """Banded conv blocks [128, 5, 118]: W[kk, c, m] = g[in - out + pad]."""
    g = _norm_fp16_taps(_gaussian(k)).astype(np.float32)
    p = k // 2
    W = np.zeros((128, NCH, 118), np.float32)
    kk = np.arange(128)
    for c, s in enumerate(STARTS):
        m = np.arange(MSZ[c])
        j = (s + kk[:, None]) - (118 * c + m[None, :]) + p
        valid = (j >= 0) & (j < k) & (kk[:, None] < KSZ[c])
        W[:, c, : MSZ[c]][valid] = g[np.clip(j, 0, k - 1)][valid]
    return W


def _overlap_planes(pl):
    """[12, 512, 512] fp32 -> [12, 128, 5*512] fp16 overlapped h-window tiles."""
    t = np.zeros((N_PLANES, NCH, 128, 512), np.float32)
    for c, s in enumerate(STARTS):
        t[:, c, : KSZ[c], :] = pl[:, s : s + KSZ[c], :]
    return np.ascontiguousarray(
        t.transpose(0, 2, 1, 3).reshape(N_PLANES, 128, NCH * 512)
    ).astype(np.float16)


_PROGRAM = {}
_SSIM_OPS = {}

# Chebyshev-centred constants for the 1-NR fast reciprocal (seed interval
# [-4.5,-4] after the BITWISE_NOT exponent flip; s1 centres the 1-NR error
# band at +-0.17%).
_FMR_S0 = -0.23549792
_FMR_S1 = 2.0017324


def _register_ssim_ops():
    """Register two fused custom DVE ops:
       ND : out = (Src0 + s0 - Src1) * (Src1 + s1)
       FMR: out = Src1 * recip_1nr(Src0); accum_out = column sum of out
    Registration appends to the concourse custom-op table (free rows exist;
    the per-NEFF DVE table is built from used ops at compile time)."""
    if _SSIM_OPS:
        return _SSIM_OPS
    from operator import add as _add
    from concourse import dve_ops as DO
    from concourse.dve_spec import AluOp, Bin, Spec, Src0, Src1, C0, C1 as SC1
    from concourse.dve_uop import DveOpSpec

    def _register(name, spec):
        if name in DO._SUB_OPCODE_FOR_NAME:
            return next(o for o in DO.OPS if o.name == name)
        row = DO._CUSTOM_DVE_ROW_BASE + len(DO.OPS)
        assert row < 0x20, "custom DVE opcode rows exhausted"
        shas = {}
        for ver in ("v3", "v4"):
            tmp = DveOpSpec(
                name=name, opcode=row,
                uops=DO.lower(spec, ver=ver),
                rd1_en=DO.has_src1(spec),
            )
            shas[ver] = tmp.sha(ver)
        op = DO.DveOp(name, spec, subdim=False, uops_sha=shas)
        DO.OPS.append(op)
        DO.CUSTOM_DVE_SPECS[name] = spec
        DO._SUB_OPCODE_FOR_NAME[name] = row
        return op

    nd_spec = Spec(
        body=(Src0 + C0 - Src1) * (Src1 + SC1),
        reference=lambda in0, in1, s0, s1, imm2: (
            (in0.astype(np.float32) + s0 - in1) * (in1 + s1)
        ).astype(np.float32),
    )

    _nx = Bin(AluOp.BITWISE_NOT, Src0, Src0)
    _y0 = _nx * C0
    _y1 = _y0 * (SC1 - Src0 * _y0)

    def _ref_fmr(in0, in1, s0, s1, imm2):
        x = in0.astype(np.float32)
        nx = (~x.view(np.int32)).view(np.float32)
        y0 = nx * np.float32(s0)
        y1 = (y0 * (np.float32(s1) - x * y0)).astype(np.float32)
        b = (y1 * in1.astype(np.float32)).astype(np.float32)
        return b, b.reshape(b.shape[0], -1).sum(axis=-1, keepdims=True)

    fmr_spec = Spec(body=_y1 * Src1, accum=_add, reference=_ref_fmr)

    _SSIM_OPS["nd"] = _register("SSIM_ND_ANT", nd_spec)
    _SSIM_OPS["fmr"] = _register("SSIM_FMR_ANT", fmr_spec)
    return _SSIM_OPS


def _build_program():
    import concourse.bass as bass
    import concourse.mybir as mybir
    from concourse import bacc, tile

    f32 = mybir.dt.float32
    f16 = mybir.dt.float16
    Act = mybir.ActivationFunctionType

    ops = _register_ssim_ops()
    nd_op = ops["nd"]
    fmr_op = ops["fmr"]

    nc = bacc.Bacc(None, target_bir_lowering=False)
    in_d = nc.dram_tensor("sduv", [N_PLANES, 128, NMAPS * FREE], f16, kind="ExternalInput")
    wb_d = nc.dram_tensor("wb", [128, 2, NCH, 118], f16, kind="ExternalInput")
    out_d = nc.dram_tensor("out", [128, OUTW], f32, kind="ExternalOutput")

    SQH = float(np.sqrt(0.5))
    COMBOS = ((0, 1), (1, 0), (1, 1))  # (colblur tap, rowblur tap)

    ev_ctr = [0]  # stage-1 eviction round-robin counter
    q0_ctr = [0]  # q0 round-robin counter
    pair_state = {}  # shared num/den tiles for the ci=0/1 fmr pairing

    with tile.TileContext(nc) as tc:
        with (
            tc.tile_pool(name="const", bufs=1) as cpool,
            tc.tile_pool(name="inp", bufs=3) as ipool,
            tc.tile_pool(name="cm", bufs=2) as cmpool,
            tc.tile_pool(name="win", bufs=6) as wpool,
            tc.tile_pool(name="ps1", bufs=2, space="PSUM") as ps1pool,
            tc.tile_pool(name="tail", bufs=1, space="PSUM") as tailpool,
            tc.tile_pool(name="ps2", bufs=3, space="PSUM") as ps2pool,
        ):
            wb = cpool.tile([128, 2, NCH, 118], f16, tag="wb")
            nc.sync.dma_start(wb[:], wb_d[:])
            wr = [wb[:, 0], wb[:, 1]]
            ocols = cpool.tile([128, OUTW], f32, tag="ocols")
            nc.vector.memset(ocols[:], 0.0)

            # dummy matmul: absorb wb DMA wait on PE once
            dummy = ps2pool.tile([128, 512], f32, tag="ps2", name="dummy")
            nc.tensor.matmul(
                dummy[0:118, 0:118], wb[0:128, 0, 0, 0:118], wb[0:128, 0, 0, 0:118],
                start=True, stop=True,
            )

            inps = [None, None]  # per-plane input tiles (bufs=2 rotation)
            cms = [None, None]   # per-plane cm tiles

            def emit_stage1_map(p, u, mp, tail):
                """Column conv for map mp, W-window u, plane p."""
                inp = inps[p % 2]
                cm = cms[p % 2]
                Kw = KSZ[u]
                ws = STARTS[u]
                mbase = mp * FREE
                # [4 c-chunks, 2 taps, 118] padded to 128 so each matmul's
                # [2, 118] output sits in a 256-float half-bank (2 banks total)
                ps = ps1pool.tile([128, 4, 2, 128], f32, tag="ps1")
                for ci_, c in enumerate((0, 1, 2, 3)):
                    Kc = KSZ[c]
                    lhs = inp[0:Kc, mbase + 512 * c + ws : mbase + 512 * c + ws + Kw]
                    nc.tensor.matmul(
                        ps[0:Kw, ci_, :, 0:118],
                        lhs, wb[0:Kc, :, c, 0:118],
                        start=True, stop=True,
                    )
                # tail chunk c=4 (H rows 472..511) into the shared tail tile
                Kc = KSZ[4]
                lhs = inp[0:Kc, mbase + 512 * 4 + ws : mbase + 512 * 4 + ws + Kw]
                nc.tensor.matmul(
                    tail[0:Kw, mp, :, 0:40],
                    lhs, wb[0:Kc, :, 4, 0:40],
                    start=True, stop=True,
                )
                # evict all 4 c-chunks x 2 taps in one op
                dst = cm[0:Kw, mp, :, 512 * u : 512 * u + 472]
                dst = dst.rearrange("p t (c j) -> p t c j", c=4)
                src = ps[0:Kw, :, :, 0:118].transpose([0, 2, 1, 3])
                if ev_ctr[0] % EV_DVE_EVERY == EV_DVE_EVERY - 1:
                    nc.vector.tensor_copy(dst, src)
                else:
                    nc.scalar.copy(dst, src)
                ev_ctr[0] += 1

            def emit_stage1_tail_evict(p, u, tail):
                cm = cms[p % 2]
                Kw = KSZ[u]
                nc.scalar.copy(
                    cm[0:Kw, :, :, 512 * u + 472 : 512 * u + 512],
                    tail[0:Kw],
                )

            def emit_stage2_group(p, ci, u):
                """Row conv + window math for combo ci, W-band u, plane p."""
                cm = cms[p % 2]
                srctap, rowtap = COMBOS[ci]
                Kw, Mu = KSZ[u], MSZ[u]
                pss = []
                for mp in range(NMAPS):
                    ps = ps2pool.tile([128, 512], f32, tag="ps2", name="ps2")
                    nc.tensor.matmul(
                        ps[0:Mu, :],
                        wr[rowtap][0:Kw, u, 0:Mu],
                        cm[0:Kw, mp, srctap, 512 * u : 512 * u + 512],
                        start=True, stop=True,
                    )
                    pss.append(ps)
                S, D, Up, Vp = pss

                sa = wpool.tile([128, 512], f16, tag="sa")
                sb = wpool.tile([128, 512], f16, tag="sb")
                nc.scalar.activation(sa[0:Mu, :], S[0:Mu, :], Act.Square, scale=SQH)
                nc.scalar.activation(sb[0:Mu, :], D[0:Mu, :], Act.Square, scale=SQH)

                a0 = wpool.tile([128, 512], f16, tag="a0")
                q0 = wpool.tile([128, 512], f16, tag="q0")
                nc.gpsimd.tensor_sub(a0[0:Mu, :], sa[0:Mu, :], sb[0:Mu, :])
                if q0_ctr[0] % Q0_DVE_EVERY == Q0_DVE_EVERY - 1:
                    nc.vector.tensor_add(q0[0:Mu, :], sa[0:Mu, :], sb[0:Mu, :])
                else:
                    nc.gpsimd.tensor_add(q0[0:Mu, :], sa[0:Mu, :], sb[0:Mu, :])
                q0_ctr[0] += 1

                num = wpool.tile([128, 512], f16, tag="num")
                den = wpool.tile([128, 512], f32, tag="den")
                nc.vector._custom_dve(
                    nd_op, out=num[0:Mu, :], in0=Vp[0:Mu, :],
                    in1=a0[0:Mu, :], s0=C2, s1=C1)
                nc.vector._custom_dve(
                    nd_op, out=den[0:Mu, :], in0=Up[0:Mu, :],
                    in1=q0[0:Mu, :], s0=C2, s1=C1)

                # fused: scratch = num * recip_1nr(den); ocols[:,t] = its col sum
                t = p * 15 + ci * 5 + u
                scratch = wpool.tile([128, 512], f16, tag="scr")
                nc.vector._custom_dve(
                    fmr_op, out=scratch[0:Mu, :], in0=den[0:Mu, :],
                    in1=num[0:Mu, :], s0=_FMR_S0, s1=_FMR_S1,
                    accum_out=ocols[0:Mu, t : t + 1])

            # software pipeline: stage-1 of plane p interleaved with
            # stage-2 of plane p-1.
            for p in range(N_PLANES + 1):
                if p < N_PLANES:
                    inp = ipool.tile([128, NMAPS * FREE], f16, tag="inp")
                    nc.sync.dma_start(inp[:], in_d[p])
                    inps[p % 2] = inp
                    cms[p % 2] = cmpool.tile([128, NMAPS, 2, FREE], f16, tag="cm", name="cm")
                for u in range(NCH):
                    tail = (
                        tailpool.tile([128, NMAPS, 2, 40], f32, tag="tail", name="tail")
                        if p < N_PLANES else None
                    )
                    if p > 0:
                        emit_stage2_group(p - 1, 0, u)
                    if p < N_PLANES:
                        emit_stage1_map(p, u, 0, tail)
                        emit_stage1_map(p, u, 1, tail)
                    if p > 0:
                        emit_stage2_group(p - 1, 1, u)
                    if p < N_PLANES:
                        emit_stage1_map(p, u, 2, tail)
                        emit_stage1_map(p, u, 3, tail)
                    if p > 0:
                        emit_stage2_group(p - 1, 2, u)
                    if p < N_PLANES:
                        emit_stage1_tail_evict(p, u, tail)

            nc.sync.dma_start(out_d[:], ocols[:])

    nc.finalize()
    return nc


def _get_program():
    global _PROGRAM
    if not isinstance(_PROGRAM, dict):
        globals()["_PROGRAM"] = {}
    if "v3" not in _PROGRAM:
        _PROGRAM["v3"] = _build_program()
    return _PROGRAM["v3"]


def _make_in_maps(img1, img2):
    x = np.asarray(img1)[:, :3].astype(np.float32)
    y = np.asarray(img2)[:, :3].astype(np.float32)
    wb = np.stack([_wblocks(5), _wblocks(11)], axis=1).astype(np.float16)
    in_maps = []
    for i in range(8):
        xs = x[4 * i : 4 * i + 4].reshape(N_PLANES, 512, 512)
        ys = y[4 * i : 4 * i + 4].reshape(N_PLANES, 512, 512)
        mS = _overlap_planes(xs + ys)
        mD = _overlap_planes(xs - ys)
        mU = _overlap_planes(xs * xs + ys * ys)
        mV = _overlap_planes(2.0 * xs * ys)
        sduv = np.concatenate([mS, mD, mU, mV], axis=2)  # [12, 128, 4*2560]
        in_maps.append({"sduv": sduv, "wb": wb})
    return in_maps


def _reduce_results(res):
    total = 0.0
    for i in range(8):
        total += np.asarray(res[i]["out"]).astype(np.float64).sum()
    npix = 32 * 3 * 512 * 512
    return np.float32(total / npix / 3.0)


def kernel(img1, img2):
    from concourse.bass_utils import run_bass_kernel_spmd

    in_maps = _make_in_maps(img1, img2)
    nc = _get_program()
    res = run_bass_kernel_spmd(nc, in_maps, core_ids=list(range(8))).results
    return _reduce_results(res)


# revision 14
# speedup vs baseline: 1.0571x; 1.0040x over previous
"""Distorted-SSIM loss kernel for Trainium2 (8 NeuronCores, data parallel).

v3 — engine-rebalanced + software-pipelined rewrite of v2 (919us).

Decomposition per [512,512] plane (x, y = img planes):
    Host precomputes 4 maps in fp16: S = x+y, D = x-y, U = x^2+y^2,
    V2 = 2xy (eliminates all on-chip prep work).
    After separable blur (col conv then row conv, both banded matmuls):
      sa = 0.5*Sb^2, sb = 0.5*Db^2          (ScalarE Act-Square from PSUM)
      a0 = sa - sb  (= 2 mu1 mu2)           (Pool)
      q0 = sa + sb  (= mu1^2 + mu2^2)       (Pool / DVE round-robin)
      num = (V2b + C2 - a0) * (a0 + C1)     (custom DVE op ND, PSUM src)
      den = (Ub  + C2 - q0) * (q0 + C1)     (custom DVE op ND, fp32 out)
      loss-col += num * recip_1nr(den)      (custom DVE op FMR: bitwise-NOT
                                             seed + 1 Newton step + mul +
                                             accumulate, one instruction)

Key changes vs v2:
  - inputs: 4 host-precomputed maps (S,D,U,V2) instead of (x,y); no
    on-chip w1/w2/xx/yy/up/vp2 passes.
  - fused FMR op: reciprocal + multiply + column-sum accumulate in one
    DVE pass (was reciprocal_approx_fast + scalar_tensor_tensor).
  - stage-1 PSUM retiled to 1-bank tiles (c-pairs) + one shared
    cross-map tail tile per u; PSUM = 3 + 1 + 4 banks = 8 exactly.
  - evictions split ScalarE/DVE, a0/q0 split Pool/DVE for engine balance.
  - stage-1 of plane p emission-interleaved with stage-2 of plane p-1 so
    the PE always has back-to-back work (HAM stays warm).
"""

import sys
import numpy as np

for _p in ("/opt/trn_rl_repo",):
    if _p not in sys.path:
        sys.path.insert(0, _p)

SIGMA = 1.5
C1 = 0.01**2
C2 = 0.03**2

STARTS = [0, 113, 231, 349, 467]
NCH = 5
KSZ = [min(128, 512 - s) for s in STARTS]
MSZ = [118, 118, 118, 118, 40]
N_PLANES = 12
FREE = NCH * 512  # 2560
NMAPS = 4
NTILES = 15 * N_PLANES  # loss columns (3 combos x 5 u per plane)
OUTW = 192  # padded

# engine-balance knobs (tuned from trace)
EV_DVE_EVERY = 3  # every k-th stage-1 evict goes to DVE instead of ScE
Q0_DVE_EVERY = 999  # every k-th q0 goes to DVE instead of Pool


def _gaussian(n, sigma=SIGMA):
    x = np.arange(n, dtype=np.float64)
    g = np.exp(-((x - n // 2) ** 2) / (2.0 * sigma**2))
    return (g / g.sum()).astype(np.float32)


def _norm_fp16_taps(g):
    """fp16 taps ULP-nudged so the fp64 sum is exactly 1.0."""
    t = g.astype(np.float16)
    for _ in range(500):
        td = t.astype(np.float64)
        err = td.sum() - 1.0
        if abs(err) < 2e-8:
            break
        bits = t.view(np.uint16).astype(np.int32) + (1 if err < 0 else -1)
        stepped = bits.astype(np.uint16).view(np.float16)
        delta = stepped.astype(np.float64) - td
        ad = np.abs(delta)
        ok = ad <= abs(err) * 1.000001
        i = int(np.argmax(np.where(ok, ad, -1.0))) if ok.any() else int(np.argmin(ad))
        t[i] = stepped[i]
    return t


def _wblocks(k):
    """Banded conv blocks [128, 5, 118]: W[kk, c, m] = g[in - out + pad]."""
    g = _norm_fp16_taps(_gaussian(k)).astype(np.float32)
    p = k // 2
    W = np.zeros((128, NCH, 118), np.float32)
    kk = np.arange(128)
    for c, s in enumerate(STARTS):
        m = np.arange(MSZ[c])
        j = (s + kk[:, None]) - (118 * c + m[None, :]) + p
        valid = (j >= 0) & (j < k) & (kk[:, None] < KSZ[c])
        W[:, c, : MSZ[c]][valid] = g[np.clip(j, 0, k - 1)][valid]
    return W


def _overlap_planes(pl):
    """[12, 512, 512] fp32 -> [12, 128, 5*512] fp16 overlapped h-window tiles."""
    t = np.zeros((N_PLANES, NCH, 128, 512), np.float32)
    for c, s in enumerate(STARTS):
        t[:, c, : KSZ[c], :] = pl[:, s : s + KSZ[c], :]
    return np.ascontiguousarray(
        t.transpose(0, 2, 1, 3).reshape(N_PLANES, 128, NCH * 512)
    ).astype(np.float16)


_PROGRAM = {}
_SSIM_OPS = {}

# Chebyshev-centred constants for the 1-NR fast reciprocal (seed interval
# [-4.5,-4] after the BITWISE_NOT exponent flip; s1 centres the 1-NR error
# band at +-0.17%).
_FMR_S0 = -0.23549792
_FMR_S1 = 2.0017324


def _register_ssim_ops():
    """Register two fused custom DVE ops:
       ND : out = (Src0 + s0 - Src1) * (Src1 + s1)
       FMR: out = Src1 * recip_1nr(Src0); accum_out = column sum of out
    Registration appends to the concourse custom-op table (free rows exist;
    the per-NEFF DVE table is built from used ops at compile time)."""
    if _SSIM_OPS:
        return _SSIM_OPS
    from operator import add as _add
    from concourse import dve_ops as DO
    from concourse.dve_spec import AluOp, Bin, Spec, Src0, Src1, C0, C1 as SC1
    from concourse.dve_uop import DveOpSpec

    def _register(name, spec):
        if name in DO._SUB_OPCODE_FOR_NAME:
            return next(o for o in DO.OPS if o.name == name)
        row = DO._CUSTOM_DVE_ROW_BASE + len(DO.OPS)
        assert row < 0x20, "custom DVE opcode rows exhausted"
        shas = {}
        for ver in ("v3", "v4"):
            tmp = DveOpSpec(
                name=name, opcode=row,
                uops=DO.lower(spec, ver=ver),
                rd1_en=DO.has_src1(spec),
            )
            shas[ver] = tmp.sha(ver)
        op = DO.DveOp(name, spec, subdim=False, uops_sha=shas)
        DO.OPS.append(op)
        DO.CUSTOM_DVE_SPECS[name] = spec
        DO._SUB_OPCODE_FOR_NAME[name] = row
        return op

    nd_spec = Spec(
        body=(Src0 + C0 - Src1) * (Src1 + SC1),
        reference=lambda in0, in1, s0, s1, imm2: (
            (in0.astype(np.float32) + s0 - in1) * (in1 + s1)
        ).astype(np.float32),
    )

    _nx = Bin(AluOp.BITWISE_NOT, Src0, Src0)
    _y0 = _nx * C0
    _y1 = _y0 * (SC1 - Src0 * _y0)

    def _ref_fmr(in0, in1, s0, s1, imm2):
        x = in0.astype(np.float32)
        nx = (~x.view(np.int32)).view(np.float32)
        y0 = nx * np.float32(s0)
        y1 = (y0 * (np.float32(s1) - x * y0)).astype(np.float32)
        b = (y1 * in1.astype(np.float32)).astype(np.float32)
        return b, b.reshape(b.shape[0], -1).sum(axis=-1, keepdims=True)

    fmr_spec = Spec(body=_y1 * Src1, accum=_add, reference=_ref_fmr)

    _SSIM_OPS["nd"] = _register("SSIM_ND_ANT", nd_spec)
    _SSIM_OPS["fmr"] = _register("SSIM_FMR_ANT", fmr_spec)
    return _SSIM_OPS


def _build_program():
    import concourse.bass as bass
    import concourse.mybir as mybir
    from concourse import bacc, tile

    f32 = mybir.dt.float32
    f16 = mybir.dt.float16
    Act = mybir.ActivationFunctionType

    ops = _register_ssim_ops()
    nd_op = ops["nd"]
    fmr_op = ops["fmr"]

    nc = bacc.Bacc(None, target_bir_lowering=False)
    in_d = nc.dram_tensor("sduv", [N_PLANES, 128, NMAPS * FREE], f16, kind="ExternalInput")
    wb_d = nc.dram_tensor("wb", [128, 2, NCH, 118], f16, kind="ExternalInput")
    out_d = nc.dram_tensor("out", [128, OUTW], f32, kind="ExternalOutput")

    SQH = float(np.sqrt(0.5))
    COMBOS = ((0, 1), (1, 0), (1, 1))  # (colblur tap, rowblur tap)

    ev_ctr = [0]  # stage-1 eviction round-robin counter
    q0_ctr = [0]  # q0 round-robin counter
    pair_state = {}  # shared num/den tiles for the ci=0/1 fmr pairing

    with tile.TileContext(nc) as tc:
        with (
            tc.tile_pool(name="const", bufs=1) as cpool,
            tc.tile_pool(name="inp", bufs=3) as ipool,
            tc.tile_pool(name="cm", bufs=2) as cmpool,
            tc.tile_pool(name="win", bufs=8) as wpool,
            tc.tile_pool(name="ps1", bufs=2, space="PSUM") as ps1pool,
            tc.tile_pool(name="tail", bufs=1, space="PSUM") as tailpool,
            tc.tile_pool(name="ps2", bufs=3, space="PSUM") as ps2pool,
        ):
            wb = cpool.tile([128, 2, NCH, 118], f16, tag="wb")
            nc.sync.dma_start(wb[:], wb_d[:])
            wr = [wb[:, 0], wb[:, 1]]
            ocols = cpool.tile([128, OUTW], f32, tag="ocols")
            nc.vector.memset(ocols[:], 0.0)

            # dummy matmul: absorb wb DMA wait on PE once
            dummy = ps2pool.tile([128, 512], f32, tag="ps2", name="dummy")
            nc.tensor.matmul(
                dummy[0:118, 0:118], wb[0:128, 0, 0, 0:118], wb[0:128, 0, 0, 0:118],
                start=True, stop=True,
            )

            inps = [None, None]  # per-plane input tiles (bufs=2 rotation)
            cms = [None, None]   # per-plane cm tiles

            def emit_stage1_map(p, u, mp, tail):
                """Column conv for map mp, W-window u, plane p."""
                inp = inps[p % 2]
                cm = cms[p % 2]
                Kw = KSZ[u]
                ws = STARTS[u]
                mbase = mp * FREE
                # [4 c-chunks, 2 taps, 118] padded to 128 so each matmul's
                # [2, 118] output sits in a 256-float half-bank (2 banks total)
                ps = ps1pool.tile([128, 4, 2, 128], f32, tag="ps1")
                for ci_, c in enumerate((0, 1, 2, 3)):
                    Kc = KSZ[c]
                    lhs = inp[0:Kc, mbase + 512 * c + ws : mbase + 512 * c + ws + Kw]
                    nc.tensor.matmul(
                        ps[0:Kw, ci_, :, 0:118],
                        lhs, wb[0:Kc, :, c, 0:118],
                        start=True, stop=True,
                    )
                # tail chunk c=4 (H rows 472..511) into the shared tail tile
                Kc = KSZ[4]
                lhs = inp[0:Kc, mbase + 512 * 4 + ws : mbase + 512 * 4 + ws + Kw]
                nc.tensor.matmul(
                    tail[0:Kw, mp, :, 0:40],
                    lhs, wb[0:Kc, :, 4, 0:40],
                    start=True, stop=True,
                )
                # evict all 4 c-chunks x 2 taps in one op
                dst = cm[0:Kw, mp, :, 512 * u : 512 * u + 472]
                dst = dst.rearrange("p t (c j) -> p t c j", c=4)
                src = ps[0:Kw, :, :, 0:118].transpose([0, 2, 1, 3])
                if ev_ctr[0] % EV_DVE_EVERY == EV_DVE_EVERY - 1:
                    nc.vector.tensor_copy(dst, src)
                else:
                    nc.scalar.copy(dst, src)
                ev_ctr[0] += 1

            def emit_stage1_tail_evict(p, u, tail):
                cm = cms[p % 2]
                Kw = KSZ[u]
                nc.scalar.copy(
                    cm[0:Kw, :, :, 512 * u + 472 : 512 * u + 512],
                    tail[0:Kw],
                )

            def emit_stage2_group(p, ci, u):
                """Row conv + window math for combo ci, W-band u, plane p."""
                cm = cms[p % 2]
                srctap, rowtap = COMBOS[ci]
                Kw, Mu = KSZ[u], MSZ[u]
                pss = []
                for mp in range(NMAPS):
                    ps = ps2pool.tile([128, 512], f32, tag="ps2", name="ps2")
                    nc.tensor.matmul(
                        ps[0:Mu, :],
                        wr[rowtap][0:Kw, u, 0:Mu],
                        cm[0:Kw, mp, srctap, 512 * u : 512 * u + 512],
                        start=True, stop=True,
                    )
                    pss.append(ps)
                S, D, Up, Vp = pss

                sa = wpool.tile([128, 512], f16, tag="sa")
                sb = wpool.tile([128, 512], f16, tag="sb")
                nc.scalar.activation(sa[0:Mu, :], S[0:Mu, :], Act.Square, scale=SQH)
                nc.scalar.activation(sb[0:Mu, :], D[0:Mu, :], Act.Square, scale=SQH)

                a0 = wpool.tile([128, 512], f16, tag="a0")
                q0 = wpool.tile([128, 512], f16, tag="q0")
                nc.gpsimd.tensor_sub(a0[0:Mu, :], sa[0:Mu, :], sb[0:Mu, :])
                if q0_ctr[0] % Q0_DVE_EVERY == Q0_DVE_EVERY - 1:
                    nc.vector.tensor_add(q0[0:Mu, :], sa[0:Mu, :], sb[0:Mu, :])
                else:
                    nc.gpsimd.tensor_add(q0[0:Mu, :], sa[0:Mu, :], sb[0:Mu, :])
                q0_ctr[0] += 1

                num = wpool.tile([128, 512], f16, tag="num")
                den = wpool.tile([128, 512], f32, tag="den")
                nc.vector._custom_dve(
                    nd_op, out=num[0:Mu, :], in0=Vp[0:Mu, :],
                    in1=a0[0:Mu, :], s0=C2, s1=C1)
                nc.vector._custom_dve(
                    nd_op, out=den[0:Mu, :], in0=Up[0:Mu, :],
                    in1=q0[0:Mu, :], s0=C2, s1=C1)

                # fused: scratch = num * recip_1nr(den); ocols[:,t] = its col sum
                t = p * 15 + ci * 5 + u
                scratch = wpool.tile([128, 512], f16, tag="scr")
                nc.vector._custom_dve(
                    fmr_op, out=scratch[0:Mu, :], in0=den[0:Mu, :],
                    in1=num[0:Mu, :], s0=_FMR_S0, s1=_FMR_S1,
                    accum_out=ocols[0:Mu, t : t + 1])

            # software pipeline: stage-1 of plane p interleaved with
            # stage-2 of plane p-1.
            for p in range(N_PLANES + 1):
                if p < N_PLANES:
                    inp = ipool.tile([128, NMAPS * FREE], f16, tag="inp")
                    nc.sync.dma_start(inp[:], in_d[p])
                    inps[p % 2] = inp
                    cms[p % 2] = cmpool.tile([128, NMAPS, 2, FREE], f16, tag="cm", name="cm")
                for u in range(NCH):
                    tail = (
                        tailpool.tile([128, NMAPS, 2, 40], f32, tag="tail", name="tail")
                        if p < N_PLANES else None
                    )
                    if p > 0:
                        emit_stage2_group(p - 1, 0, u)
                    if p < N_PLANES:
                        emit_stage1_map(p, u, 0, tail)
                        emit_stage1_map(p, u, 1, tail)
                    if p > 0:
                        emit_stage2_group(p - 1, 1, u)
                    if p < N_PLANES:
                        emit_stage1_map(p, u, 2, tail)
                        emit_stage1_map(p, u, 3, tail)
                    if p > 0:
                        emit_stage2_group(p - 1, 2, u)
                    if p < N_PLANES:
                        emit_stage1_tail_evict(p, u, tail)

            nc.sync.dma_start(out_d[:], ocols[:])

    nc.finalize()
    return nc


def _get_program():
    global _PROGRAM
    if not isinstance(_PROGRAM, dict):
        globals()["_PROGRAM"] = {}
    if "v3" not in _PROGRAM:
        _PROGRAM["v3"] = _build_program()
    return _PROGRAM["v3"]


def _make_in_maps(img1, img2):
    x = np.asarray(img1)[:, :3].astype(np.float32)
    y = np.asarray(img2)[:, :3].astype(np.float32)
    wb = np.stack([_wblocks(5), _wblocks(11)], axis=1).astype(np.float16)
    in_maps = []
    for i in range(8):
        xs = x[4 * i : 4 * i + 4].reshape(N_PLANES, 512, 512)
        ys = y[4 * i : 4 * i + 4].reshape(N_PLANES, 512, 512)
        mS = _overlap_planes(xs + ys)
        mD = _overlap_planes(xs - ys)
        mU = _overlap_planes(xs * xs + ys * ys)
        mV = _overlap_planes(2.0 * xs * ys)
        sduv = np.concatenate([mS, mD, mU, mV], axis=2)  # [12, 128, 4*2560]
        in_maps.append({"sduv": sduv, "wb": wb})
    return in_maps


def _reduce_results(res):
    total = 0.0
    for i in range(8):
        total += np.asarray(res[i]["out"]).astype(np.float64).sum()
    npix = 32 * 3 * 512 * 512
    return np.float32(total / npix / 3.0)


def kernel(img1, img2):
    from concourse.bass_utils import run_bass_kernel_spmd

    in_maps = _make_in_maps(img1, img2)
    nc = _get_program()
    res = run_bass_kernel_spmd(nc, in_maps, core_ids=list(range(8))).results
    return _reduce_results(res)
